# revision 49
# baseline (speedup 1.0000x reference)
"""BiGCN (2-layer bidirectional GCN + global add pool) on 8 Trainium2 NeuronCores.

Strategy (hardcoded for the nn_BiGCN_graphcl problem shapes):
  - Nodes are sharded graph-aligned: core c owns graphs [128c, 128c+128) and
    their (contiguous, batch-sorted) node range, padded to a common NPC.
  - Per direction (td / bu), edges are assigned to the core owning their
    target node.  GCNConv is computed as
        out = dinv * (scatter_add(hn[src], dst) + hn) + b,   hn = dinv * (x @ W)
    so no per-edge scaling is needed on device.
  - A combined fp8_e4m3 message table ([8*NPC, 256], row v = [hn_td[v] |
    hn_bu[v]]) is AllGathered per layer in window chunks; each core gathers
    256B rows for its edge shard with dma_gather, builds one-hots with a DVE
    is_equal against an iota constant, and segment-sums on the TensorEngine
    into per-window (128-node) PSUM tiles.  fp8 messages cost ~2.5e-3 final
    relative error (validated against the fp32 reference).
  - Edges are grouped per (target-super of 8 windows, source table block);
    within a group slots are window-sorted with trailing padding (pads
    gather block row 0 and carry dloc=-1 so their one-hot column is zero).
    Q7 descriptor generation on the GpSimd engine is the kernel bottleneck
    (~3.6ns/edge, serial), so padding is minimized (~4%) and gather queues
    are load-balanced.  The one-hot is built per (group128, window) pair; a
    slot group straddling a window boundary gets one matmul per window.
  - AllGather emission is ordered so its input-side waits never block the
    in-order GpSimd queue: layer 1 processes super 0 last, layer 2 needs
    table chunk 0 last (per-super block rotation), and mid-layer AllGathers
    are emitted one super after their data completes.
  - The SPMD program is identical on all cores: all per-core variation lives
    in uploaded index/data tensors.
  - Graph pooling is a second one-hot matmul into a [128 graphs, 2*128] PSUM
    tile; the host just concatenates the 8 per-core [128, 256] outputs.
"""

import math
import numpy as np
import ml_dtypes

BF16 = ml_dtypes.bfloat16

# ---------------------------------------------------------------- problem cfg
FULL_CFG = dict(
    N=100000, E=1600000, IN_FEATS=256, HIDDEN=128, OUT_FEATS=128,
    NUM_GRAPHS=1024, N_CORES=8, SW=8,
)

# pad-slot index value: -1 enables the Q7 trailing-trim fast path; 0 gathers
# block row 0 (safe fallback, baseline behaviour)
PAD_IDX = 0


def _round_up(x, m):
    return (x + m - 1) // m * m


# =====================================================================
# Host-side metadata construction
# =====================================================================

def build_partition(batch, cfg, deg_td=None, deg_bu=None):
    """Graph-aligned node partition. Returns dict with per-core node ranges.

    If degree arrays are given, each core's local node order is permuted so
    that per-window (128-node) degree sums are balanced."""
    N, C, G = cfg["N"], cfg["N_CORES"], cfg["NUM_GRAPHS"]
    gpc = G // C  # graphs per core
    starts = np.searchsorted(batch, np.arange(0, G + 1, gpc))
    counts = np.diff(starts)
    NPC = max(128, _round_up(int(counts.max()), 128))
    W = NPC // 128
    node_core = np.searchsorted(starts[1:], np.arange(N), side="right")
    node_local = np.arange(N) - starts[node_core]

    if deg_td is not None:
        for c in range(C):
            lo, hi = starts[c], starts[c + 1]
            cnt = hi - lo
            dt = deg_td[lo:hi].astype(np.int64)
            db = deg_bu[lo:hi].astype(np.int64)
            order = np.argsort(-(dt + db), kind="stable")
            rem_t = np.full(W, dt.sum() / W, np.float64)
            rem_b = np.full(W, db.sum() / W, np.float64)
            room = np.full(W, 128, np.int64)
            assign = np.empty(cnt, np.int64)
            for j in order:
                score = np.minimum(rem_t - dt[j], rem_b - db[j])
                score[room <= 0] = -np.inf
                w = int(np.argmax(score))
                assign[j] = w
                rem_t[w] -= dt[j]
                rem_b[w] -= db[j]
                room[w] -= 1
            slot_in_w = np.zeros(W, np.int64)
            newloc = np.empty(cnt, np.int64)
            for j in range(cnt):
                w = assign[j]
                newloc[j] = w * 128 + slot_in_w[w]
                slot_in_w[w] += 1
            node_local[lo:hi] = newloc

    # ---- table chunk decomposition: window chunks of <=32 windows so each
    # block's 128*wq*8 table rows stay within int16 index range.  The first
    # chunk is small so the first AllGather fires early in the dense phase. ----
    ws = [min(8, W)]
    rem = W - ws[0]
    while rem > 0:
        take = min(32, rem)
        ws.append(take)
        rem -= take
    NBLK = len(ws)
    cw = np.concatenate([[0], np.cumsum(ws)])
    assert cw[-1] == W

    chunk_of_w = np.searchsorted(cw[1:], np.arange(W), side="right")
    q = chunk_of_w[np.minimum(node_local // 128, W - 1)]
    rpr = 128 * np.diff(cw)  # rows per rank per chunk
    base = np.concatenate([[0], np.cumsum(rpr * C)])
    table_row = base[q] + node_core * rpr[q] + (node_local - 128 * cw[q])
    bounds = [int(b) for b in base]
    return dict(starts=starts, counts=counts, NPC=NPC, gpc=gpc, NBLK=NBLK,
                node_core=node_core.astype(np.int64),
                node_local=node_local.astype(np.int64),
                table_row=table_row.astype(np.int64),
                cw=cw, bounds=bounds)


def build_direction_meta(gather_nodes, target_nodes, part, cfg):
    """Per-core gather index / per-pair dstloc arrays and the uniform group
    structure for one edge direction.

    gather_nodes[e]: node whose table row is gathered for edge e.
    target_nodes[e]: node receiving the contribution.

    Edges are grouped per (target super, source block).  Within a group the
    slots are sorted by target window and padding is trailing-only (idx=-1,
    trimmed by the Q7 descriptor generator at runtime per core).  The one-hot
    columns are per (slot-group-of-128, window) PAIR: dloc_pair[slot%128, p]
    holds the in-window position if the slot targets that pair's window.
    """
    N, C = cfg["N"], cfg["N_CORES"]
    SW = cfg["SW"]
    NPC = part["NPC"]
    W = NPC // 128
    NS = (W + SW - 1) // SW
    NBLK = part["NBLK"]

    deg = np.bincount(target_nodes, minlength=N).astype(np.float64) + 1.0

    bounds = part["bounds"]
    assert all(bounds[i + 1] - bounds[i] <= 32768 for i in range(NBLK))
    bounds_arr = np.array(bounds[1:-1])

    tr_g = part["table_row"][gather_nodes]
    t_core = part["node_core"][target_nodes]
    t_local = part["node_local"][target_nodes]
    lw = t_local // 128          # window
    dloc = t_local % 128         # position within window
    blk = np.searchsorted(bounds_arr, tr_g, side="right")
    idxv = tr_g - np.array(bounds[:-1])[blk]
    sup = lw // SW

    nkeys = NS * NBLK
    key = sup * NBLK + blk

    # per-core sorted edge lists per (s, b): order by window
    per_core = []
    counts = np.zeros((C, nkeys), np.int64)
    for c in range(C):
        m = t_core == c
        k = key[m]
        order = np.lexsort((lw[m], k))
        ks = k[order]
        run_start = np.searchsorted(ks, np.arange(nkeys + 1))
        per_core.append(dict(order=order, run_start=run_start,
                             iv=idxv[m][order], dl=dloc[m][order],
                             w=lw[m][order]))
        counts[c] = np.diff(run_start)
    cnt_max = counts.max(axis=0)
    G_sb = np.ceil(cnt_max / 128).astype(np.int64)  # groups per (s,b)

    # pairs per (s,b): union over cores of (group, window)
    pairkeys = [set() for _ in range(nkeys)]
    for c in range(C):
        pc = per_core[c]
        rs = pc["run_start"]
        n_c = rs[-1]
        slot_in_run = np.arange(n_c) - rs[:-1].repeat(np.diff(rs))
        gk = (slot_in_run // 128) * (W + 1) + pc["w"]
        for kix in range(nkeys):
            r0, r1 = rs[kix], rs[kix + 1]
            if r1 > r0:
                pairkeys[kix].update(np.unique(gk[r0:r1]).tolist())
    struct = []
    off16 = 0
    offP = 0
    for s in range(NS):
        w_lo, w_hi = s * SW, min((s + 1) * SW, W)
        for b in range(NBLK):
            kix = s * NBLK + b
            G = int(G_sb[kix])
            pairs = sorted((k // (W + 1), k % (W + 1)) for k in pairkeys[kix])
            struct.append(dict(s=s, b=b, w_lo=w_lo, w_hi=w_hi, G=G,
                               pairs=pairs, P=len(pairs),
                               off16=off16, offP=offP))
            off16 += G * 8
            offP += len(pairs)

    # every window must receive at least one matmul (to close its psum
    # accumulation); guaranteed in practice, asserted here.
    covered = set()
    for sb in struct:
        for (g, w) in sb["pairs"]:
            covered.add(w)
    assert covered == set(range(W)), f"uncovered windows: {set(range(W)) - covered}"

    CG16 = off16
    CP = offP
    Gmax = int(G_sb.max())
    Pmax = max(sb["P"] for sb in struct)

    # per-core uploads
    idx_all = np.full((C, 16, CG16), PAD_IDX, np.int16)
    dloc_all = np.full((C, 128, CP), -1, np.int8)
    for c in range(C):
        pc = per_core[c]
        for sb in struct:
            kix = sb["s"] * NBLK + sb["b"]
            r0, r1 = pc["run_start"][kix], pc["run_start"][kix + 1]
            n = r1 - r0
            if n == 0:
                continue
            iv = pc["iv"][r0:r1]
            dl = pc["dl"][r0:r1]
            wv = pc["w"][r0:r1]
            slot = np.arange(n)
            idx_all[c, slot % 16, sb["off16"] + slot // 16] = iv.astype(np.int16)
            # pair columns via sorted-key lookup
            pk = np.array([g * (W + 1) + w for (g, w) in sb["pairs"]])
            pcol = np.searchsorted(pk, (slot // 128) * (W + 1) + wv)
            dloc_all[c, slot % 128, sb["offP"] + pcol] = dl.astype(np.int8)
    idx_all = np.tile(idx_all, (1, 8, 1))  # replicate to 128 partitions

    return dict(deg=deg, struct=struct, CG16=CG16, CP=CP, Gmax=Gmax,
                Pmax=Pmax, NS=NS, W=W, bounds=bounds,
                idx_all=idx_all, dloc_all=dloc_all)


def build_all_inputs(x, edge_index, batch, Ws, bs, cfg):
    """Produce per-core in_maps plus structural metadata."""
    C = cfg["N_CORES"]
    N = cfg["N"]
    src = np.asarray(edge_index[0])
    dst = np.asarray(edge_index[1])
    part = build_partition(batch, cfg,
                           deg_td=np.bincount(dst, minlength=N),
                           deg_bu=np.bincount(src, minlength=N))
    NPC = part["NPC"]
    W = NPC // 128

    td = build_direction_meta(src, dst, part, cfg)   # gather src row, scatter to dst
    bu = build_direction_meta(dst, src, part, cfg)   # reversed

    Pmax = max(td["Pmax"], bu["Pmax"])
    iota_rep = np.tile(np.arange(128, dtype=np.int8), Pmax)[None, :].repeat(128, 0)

    # per-core tensors
    in_maps = []
    xT_full = np.ascontiguousarray(np.asarray(x).T)  # [IN, N]
    batch_np = np.asarray(batch)
    for c in range(C):
        lo, hi = part["starts"][c], part["starts"][c + 1]
        li = part["node_local"][lo:hi]
        xT = np.zeros((cfg["IN_FEATS"], NPC), BF16)
        xT[:, li] = xT_full[:, lo:hi].astype(BF16)
        deg_t = np.ones((128, W), np.float32)
        deg_b = np.ones((128, W), np.float32)
        deg_t[li % 128, li // 128] = td["deg"][lo:hi].astype(np.float32)
        deg_b[li % 128, li // 128] = bu["deg"][lo:hi].astype(np.float32)
        bl = np.full((128, W), -1, np.int8)
        bl[li % 128, li // 128] = (batch_np[lo:hi] - c * part["gpc"]).astype(np.int8)
        im = dict(
            xT=xT, ident=np.eye(128, dtype=BF16),
            deg_td=deg_t, deg_bu=deg_b, batchloc=bl, iota_rep=iota_rep,
            idx_td=td["idx_all"][c], idx_bu=bu["idx_all"][c],
            dstloc_td=td["dloc_all"][c], dstloc_bu=bu["dloc_all"][c],
            W_td1=Ws[0].astype(BF16), W_bu1=Ws[2].astype(BF16),
            W_td2=Ws[1].astype(BF16), W_bu2=Ws[3].astype(BF16),
            b_td1=np.tile(bs[0][None, :], (128, 1)).astype(np.float32),
            b_td2=np.tile(bs[1][None, :], (128, 1)).astype(np.float32),
            b_bu1=np.tile(bs[2][None, :], (128, 1)).astype(np.float32),
            b_bu2=np.tile(bs[3][None, :], (128, 1)).astype(np.float32),
        )
        in_maps.append(im)
    meta = dict(part=part, td=td, bu=bu, Pmax=Pmax, NPC=NPC, W=W, cfg=cfg)
    return in_maps, meta


# =====================================================================
# Bass program
# =====================================================================

def build_bass(meta):
    import concourse.bacc as bacc
    import concourse.mybir as mybir
    import concourse.tile as tile

    cfg = meta["cfg"]
    C = cfg["N_CORES"]
    NPC, W, Pmax = meta["NPC"], meta["W"], meta["Pmax"]
    IN, HID = cfg["IN_FEATS"], cfg["HIDDEN"]
    NBLK = meta["part"]["NBLK"]
    Gmax = max(meta["td"]["Gmax"], meta["bu"]["Gmax"])
    f32, bf16, i16 = mybir.dt.float32, mybir.dt.bfloat16, mybir.dt.int16
    f8 = mybir.dt.float8e4
    i8 = mybir.dt.int8

    nc = bacc.Bacc("TRN2", target_bir_lowering=False, debug=False, num_devices=C,
                   num_swdge_queues=4)

    # ---- I/O ----
    ten = {}
    def inp(name, shape, dt):
        ten[name] = nc.dram_tensor(name, shape, dt, kind="ExternalInput")
        return ten[name]

    inp("xT", [IN, NPC], bf16)
    inp("deg_td", [128, W], f32); inp("deg_bu", [128, W], f32)
    inp("batchloc", [128, W], i8)
    inp("iota_rep", [128, Pmax * 128], i8)
    inp("ident", [128, 128], bf16)
    for d in ("td", "bu"):
        m = meta[d]
        inp(f"idx_{d}", [128, m["CG16"]], i16)
        inp(f"dstloc_{d}", [128, m["CP"]], i8)
        inp(f"W_{d}1", [IN, HID], bf16)
        inp(f"W_{d}2", [HID, HID], bf16)
        inp(f"b_{d}1", [128, HID], f32)
        inp(f"b_{d}2", [128, HID], f32)
    out_t = nc.dram_tensor("out", [128, 2 * HID], f32, kind="ExternalOutput")

    # internal DRAM: combined fp8 [td|bu] AG inputs + tables, per layer
    ag_in, table = {}, {}
    for l in (1, 2):
        ag_in[l] = nc.dram_tensor(f"agin_{l}", [NPC, 2 * HID], f8, kind="Internal")
        table[l] = nc.dram_tensor(f"table_{l}", [C * NPC, 2 * HID], f8,
                                  kind="Internal", addr_space="Shared")
    dummy_dram = nc.dram_tensor("dummy_rgn", [128, 2 * HID], f8, kind="Internal")

    rg = [list(range(C))]

    from contextlib import ExitStack
    with tile.TileContext(nc) as tc, ExitStack() as stack:
        def pool(name, bufs, space="SBUF"):
            return stack.enter_context(tc.tile_pool(name=name, bufs=bufs, space=space))

        const = pool("const", 1)
        xt_p = pool("xt", 4)
        hn_p = pool("hn", 4)                 # hn tiles to DRAM
        idx_p = pool("idx", 12)
        dl_p = pool("dl", 12)
        gat_p = pool("gat", 8)               # gathered edge tiles
        oh_p = pool("oh", 4)                 # one-hot tiles
        win_p = pool("win", 6, "PSUM")       # window psum, 4 windows/bank
        epi_p = pool("epi", 6)               # epilogue sbuf tiles
        h1_p = pool("h1", 4)
        t_p = pool("tt", 4)                  # transposes
        po_p = pool("po", 4)                 # pool one-hot
        outp = pool("outp", 1)
        hps_cm = tc.tile_pool(name="hps", bufs=2, space="PSUM")
        hps_p = hps_cm.__enter__()

        # ---- warmup: trigger the Q7 extended-inst library reload early and
        # zero the gather pool buffers so trimmed (never-written) slots hold
        # benign values for the one-hot matmuls. ----
        dummy_idx = const.tile([128, 8], i16, tag="dummy_idx")
        nc.vector.memset(dummy_idx[:], 0)
        dummy_gt = const.tile([128, 1, 2 * HID], f8, tag="dummy_gt")
        nc.gpsimd.dma_gather(dummy_gt[:], dummy_dram[:, :], dummy_idx[:],
                             num_idxs=128, num_idxs_reg=128, elem_size=2 * HID,
                             single_packet=False, queue_num=0)
        gat_warm = []
        for i in range(8):
            t = gat_p.tile([128, Gmax, 2 * HID], f8, tag="gat", name=f"gwarm_{i}")
            nc.vector.memset(t[:], 0.0)
            gat_warm.append(t)

        # ---- constants in SBUF ----
        iota = const.tile([128, Pmax * 128], i8, tag="iota")
        nc.sync.dma_start(iota[:], ten["iota_rep"][:])
        Wt = {}
        for d in ("td", "bu"):
            for l, k in ((1, IN), (2, HID)):
                chunks = []
                for kk in range(k // 128):
                    t = const.tile([128, HID], bf16, tag=f"W_{d}{l}_{kk}", name=f"W_{d}{l}_{kk}")
                    nc.sync.dma_start(t[:], ten[f"W_{d}{l}"][kk * 128:(kk + 1) * 128, :])
                    chunks.append(t)
                Wt[d, l] = chunks
        bt = {}
        for d in ("td", "bu"):
            for l in (1, 2):
                t = const.tile([128, HID], f32, tag=f"b_{d}{l}", name=f"bt_{d}{l}")
                nc.sync.dma_start(t[:], ten[f"b_{d}{l}"][:])
                bt[d, l] = t
        zrow = const.tile([1, 512], bf16, tag="zrow")
        nc.gpsimd.memset(zrow[:], 0.0)
        ident = const.tile([128, 128], bf16, tag="ident")
        nc.sync.dma_start(ident[:], ten["ident"][:])
        batchloc = const.tile([128, W], i8, tag="batchloc")
        nc.sync.dma_start(batchloc[:], ten["batchloc"][:])

        dinv = {}
        for d in ("td", "bu"):
            degt = const.tile([128, W], f32, tag=f"deg_{d}", name=f"degt_{d}")
            nc.sync.dma_start(degt[:], ten[f"deg_{d}"][:])
            rec = const.tile([128, W], f32, tag=f"rec_{d}", name=f"rec_{d}")
            nc.vector.reciprocal(rec[:], degt[:])
            dv = const.tile([128, W], f32, tag=f"dinv_{d}", name=f"dinv_{d}")
            nc.scalar.activation(dv[:], rec[:], mybir.ActivationFunctionType.Sqrt)
            dinv[d] = dv

        # ---- phase A1: conv1 tables (both directions share xT loads) ----
        cw = meta["part"]["cw"]
        bounds = meta["td"]["bounds"]

        def emit_ag(l, q):
            # NOTE: collectives are only legal on the Pool (GpSimd) or DMA
            # engines on TRN2, and the issuing queue blocks until the
            # collective completes — emission points below are chosen so the
            # ~45us-per-AllGather queue stalls overlap work that does not
            # depend on them.
            nc.gpsimd.collective_compute(
                "AllGather", mybir.AluOpType.bypass, replica_groups=rg,
                ins=[ag_in[l][128 * int(cw[q]):128 * int(cw[q + 1]), :]],
                outs=[table[l][bounds[q]:bounds[q + 1], :]])

        nK = IN // 128
        for w0 in range(0, W, 4):
            wn = min(4, W - w0)
            xts = []
            for kk in range(nK):
                t = xt_p.tile([128, 512], bf16, tag="xt", name=f"xt_{w0}_{kk}")
                nc.sync.dma_start(t[:, :wn * 128],
                                  ten["xT"][kk * 128:(kk + 1) * 128,
                                            w0 * 128:(w0 + wn) * 128])
                xts.append(t)
            for d in ("td", "bu"):
                hps = hps_p.tile([128, 512], f32, tag="hps")
                for w in range(w0, w0 + wn):
                    o = (w - w0) * 128
                    for kk in range(nK):
                        nc.tensor.matmul(hps[:, o:o + 128], xts[kk][:, o:o + 128],
                                         Wt[d, 1][kk][:],
                                         start=(kk == 0), stop=(kk == nK - 1))
                hn8 = hn_p.tile([128, 512], f8, tag="hn")
                nc.vector.tensor_tensor(
                    out=hn8[:, :wn * 128],
                    in0=hps[:, :wn * 128].rearrange("p (j f) -> p j f", f=128),
                    in1=dinv[d][:, w0:w0 + wn].rearrange("p (j o) -> p j o", o=1)
                        .to_broadcast([128, wn, 128]),
                    op=mybir.AluOpType.mult)
                off = 0 if d == "td" else HID
                nc.sync.dma_start(
                    ag_in[1][w0 * 128:(w0 + wn) * 128, off:off + HID]
                        .rearrange("(j p) f -> p j f", p=128),
                    hn8[:, :wn * 128].rearrange("p (j f) -> p j f", f=128))
            # layer-1 AllGathers are emitted lazily in the edge phase, right
            # before the first gather reading each block, so the first
            # gathers are not queued behind serial collective executions

        # ---- edge phase for one conv ----
        AHEAD = 5
        # layer-1 table chunks' AllGathers are emitted lazily, just before the
        # first gather that reads each block, so their execution and input
        # waits interleave with useful gathers in the in-order GpSimd queue
        ag1_emitted = set()

        def edge_phase(d, l):
            m = meta[d]
            NS = m["NS"]
            # emission order: layer 1 processes super 0 LAST so table chunk 0
            # (the small one) is the last chunk-2 AllGather dependency; layer 2
            # processes block 0 last in each super so that AllGather's tail
            # hides behind the other blocks' gathers.
            sup_order = list(range(NS)) if l == 2 else ([*range(1, NS), 0] if NS > 1 else [0])
            blk_order = list(range(NBLK)) if l == 1 else [*range(1, NBLK), 0]
            structs = [m["struct"][s * NBLK + b] for s in sup_order for b in blk_order]
            # last (emission_idx, pair_idx) per window for stop flags
            last_mm = {}
            for sbi, sb in enumerate(structs):
                for pi, (g, w) in enumerate(sb["pairs"]):
                    last_mm[w] = (sbi, pi)
            quad_tiles = {}
            def win_ap(w):
                q = w // 4
                if q not in quad_tiles:
                    qt = win_p.tile([128, 512], f32, tag="win",
                                    name=f"win_{d}{l}_{q}")
                    nc.tensor.matmul(qt[:], zrow[0:1, 0:128], zrow[0:1, 0:512],
                                     start=True, stop=False, skip_group_check=True)
                    quad_tiles[q] = qt
                return quad_tiles[q][:, (w % 4) * 128:(w % 4 + 1) * 128]
            loaded = {}
            def load(i):
                sb = structs[i]
                G, P = sb["G"], sb["P"]
                if G == 0:
                    loaded[i] = None
                    return
                it = idx_p.tile([128, G * 8], i16, tag="idx")
                nc.scalar.dma_start(it[:], ten[f"idx_{d}"][:, sb["off16"]:sb["off16"] + G * 8])
                dlt = dl_p.tile([128, P], i8, tag="dl")
                nc.scalar.dma_start(dlt[:], ten[f"dstloc_{d}"][:, sb["offP"]:sb["offP"] + P])
                loaded[i] = (it, dlt)
            for i in range(min(AHEAD, len(structs))):
                load(i)
            for sbi, sb in enumerate(structs):
                if sbi + AHEAD < len(structs):
                    load(sbi + AHEAD)
                if l == 1 and sb["b"] not in ag1_emitted:
                    emit_ag(1, sb["b"])
                    ag1_emitted.add(sb["b"])
                G, P = sb["G"], sb["P"]
                if G > 0:
                    it, dlt = loaded.pop(sbi)
                    gt = gat_p.tile([128, G, 2 * HID], f8, tag="gat")
                    blk = table[l][m["bounds"][sb["b"]]:m["bounds"][sb["b"] + 1], :]
                    # queue with least outstanding descriptor load
                    qsel = int(np.argmin(qload))
                    qload[qsel] += G
                    nc.gpsimd.dma_gather(gt[:], blk, it[:], num_idxs=G * 128,
                                         num_idxs_reg=G * 128, elem_size=2 * HID,
                                         single_packet=False, queue_num=qsel)
                    oh = oh_p.tile([128, P * 128], f8, tag="oh")
                    nc.vector.tensor_tensor(
                        out=oh[:],
                        in0=dlt[:].rearrange("p (g o) -> p g o", o=1).to_broadcast([128, P, 128]),
                        in1=iota[:, :P * 128].rearrange("p (g f) -> p g f", f=128),
                        op=mybir.AluOpType.is_equal)
                    hoff = 0 if d == "td" else HID
                    for pi, (g, w) in enumerate(sb["pairs"]):
                        nc.tensor.matmul(
                            win_ap(w)[:], oh[:, pi * 128:(pi + 1) * 128],
                            gt[:, g, hoff:hoff + HID],
                            start=False, stop=(last_mm[w] == (sbi, pi)),
                            skip_group_check=True)
                # epilogues for completed supers: after last block of super
                if sb["b"] == blk_order[-1]:
                    for w in range(sb["w_lo"], sb["w_hi"]):
                        epilogue(d, l, w, win_ap(w))
                    quad_tiles.clear()
                    yield sb["s"]
                else:
                    yield None

        def epilogue(d, l, w, pt):
            hoff = 0 if d == "td" else HID
            hn = hn_p.tile([128, HID], f8, tag="hn_ep")
            nc.scalar.dma_start(hn[:], ag_in[l][w * 128:(w + 1) * 128, hoff:hoff + HID])
            o1 = epi_p.tile([128, HID], f32, tag="o1")
            nc.vector.scalar_tensor_tensor(
                out=o1[:], in0=pt[:], scalar=dinv[d][:, w:w + 1], in1=bt[d, l][:],
                op0=mybir.AluOpType.mult, op1=mybir.AluOpType.add)
            o2 = epi_p.tile([128, HID], bf16, tag="o2")
            nc.vector.scalar_tensor_tensor(
                out=o2[:], in0=hn[:], scalar=dinv[d][:, w:w + 1], in1=o1[:],
                op0=mybir.AluOpType.mult, op1=mybir.AluOpType.add)
            if l == 1:
                h1 = h1_p.tile([128, HID], bf16, tag="h1")
                nc.scalar.activation(h1[:], o2[:], mybir.ActivationFunctionType.Relu)
                tps = hps_p.tile([128, HID], bf16, tag="hps", name=f"tps_{d}_{w}")
                nc.tensor.transpose(tps[:], h1[:], ident[:])
                h1T = t_p.tile([128, HID], bf16, tag="h1T")
                nc.vector.tensor_copy(h1T[:], tps[:])
                h2 = hps_p.tile([128, HID], f32, tag="hps")
                nc.tensor.matmul(h2[:], h1T[:], Wt[d, 2][0][:], start=True, stop=True)
                hn2 = hn_p.tile([128, HID], f8, tag="hn2")
                nc.vector.tensor_scalar_mul(hn2[:], h2[:], dinv[d][:, w:w + 1])
                nc.sync.dma_start(ag_in[2][w * 128:(w + 1) * 128, hoff:hoff + HID], hn2[:])
            else:
                po = po_p.tile([128, 128], bf16, tag="po")
                nc.vector.tensor_tensor(
                    out=po[:],
                    in0=batchloc[:, w:w + 1].to_broadcast([128, 128]),
                    in1=iota[:, :128],
                    op=mybir.AluOpType.is_equal)
                off = 0 if d == "td" else HID
                nc.tensor.matmul(pool_psum_t[:, off:off + HID], po[:], o2[:],
                                 start=False, stop=(w == W - 1),
                                 skip_group_check=True)

        qload = np.zeros(4, np.int64)

        SW = cfg["SW"]
        NS_all = (W + SW - 1) // SW
        chunk_sups = [set(range(int(cw[q]) // SW, (int(cw[q + 1]) + SW - 1) // SW))
                      for q in range(NBLK)]
        l1_order = [*range(1, NS_all), 0] if NS_all > 1 else [0]
        pos_of = {s: i for i, s in enumerate(l1_order)}
        # emit chunk q's layer-2 AllGather one super AFTER its data is complete
        # (unless near the layer end) so its input wait never blocks the queue
        emit_when = {}
        for q in range(NBLK):
            lastpos = max(pos_of[s] for s in chunk_sups[q])
            delay = 1 if lastpos + 2 < NS_all else 0
            emit_when[q] = lastpos + 1 + delay

        def run_layer(l):
            gens = {"td": edge_phase("td", l), "bu": edge_phase("bu", l)}
            done = {"td": False, "bu": False}
            sup_done = {"td": set(), "bu": set()}
            emitted = [False] * NBLK
            while not all(done.values()):
                for d in ("td", "bu"):
                    if done[d]:
                        continue
                    try:
                        res = next(gens[d])
                    except StopIteration:
                        done[d] = True
                        sup_done[d] = set(range(NS_all))
                        res = None
                    if res is not None:
                        sup_done[d].add(res)
                    if l == 1:
                        both = sup_done["td"] & sup_done["bu"]
                        for q in range(NBLK):
                            if (not emitted[q] and chunk_sups[q] <= both
                                    and len(both) >= emit_when[q]):
                                emit_ag(2, q)
                                emitted[q] = True

        run_layer(1)
        hps_cm.__exit__(None, None, None)
        pool_ps = stack.enter_context(tc.tile_pool(name="plps", bufs=1, space="PSUM"))
        pool_psum_t = pool_ps.tile([128, 2 * HID], f32, tag="pool", name="pool_psum_t")
        nc.tensor.matmul(pool_psum_t[:], zrow[0:1, 0:128], zrow[0:1, 0:2 * HID],
                         start=True, stop=False, skip_group_check=True)
        run_layer(2)

        outsb = outp.tile([128, 2 * HID], f32, tag="out")
        nc.vector.tensor_copy(outsb[:], pool_psum_t[:])
        nc.sync.dma_start(out_t[:], outsb[:])

    nc.compile()
    return nc


# =====================================================================
# Entry point
# =====================================================================

def _run(inputs, cfg, trace=False):
    from concourse import bass_utils
    x = np.asarray(inputs["x"], np.float32)
    edge_index = np.asarray(inputs["edge_index"])
    batch = np.asarray(inputs["batch"])
    Ws = [np.asarray(inputs[k], np.float32) for k in ("W_td1", "W_td2", "W_bu1", "W_bu2")]
    bs = [np.asarray(inputs[k], np.float32) for k in ("b_td1", "b_td2", "b_bu1", "b_bu2")]
    in_maps, meta = build_all_inputs(x, edge_index, batch, Ws, bs, cfg)
    nc = build_bass(meta)
    res = bass_utils.run_bass_kernel_spmd(
        nc, in_maps, core_ids=list(range(cfg["N_CORES"])), trace=trace)
    gpc = meta["part"]["gpc"]
    out = np.concatenate([res.results[c]["out"][:gpc] for c in range(cfg["N_CORES"])], axis=0)
    return out.astype(np.float32), res


def kernel(**inputs):
    out, _ = _run(inputs, FULL_CFG, trace=False)
    return out
